# revision 73
# baseline (speedup 1.0000x reference)
"""MoE block (top-1 routing, shared FFN + per-expert LoRA) on 8 TRN2 NeuronCores.

Strategy: data-parallel over the 8192 tokens (1024 tokens/core), weights
replicated. The reference's dense-then-mask expert loop collapses to:

    logits = x @ gate_W.T + gate_b ; e* = argmax(logits)
    u      = x @ A_cat.T                 [N, 32]
    u_m    = u * onehot-mask(e*)  (zero all but selected expert's 4 lora rows)
    inter  = relu(x @ wi_W.T + u_m @ B_cat + wi_b)
    out    = inter @ wo_W.T + wo_b

Everything runs in transposed (feature-major) layout on chip; the host
pre-transposes the shards/weights and re-transposes the output.

Router logits/u are fp32-accurate via a split-bf16 scheme (x16+dx16 against
[Acat|g16|dg16]) so routing tie-breaks match the reference. The top-1 mask is
built without PE/DVE transpose ping-pong: logits stay expert-major [8, 512],
a GPSIMD partition_all_reduce gives the per-token max on all partitions,
is_equal gives the mask, and a tiny [8->32] replication matmul expands it to
the lora rows. All input tensors are pre-tiled on the host so every DMA is a
contiguous copy (the input stream is HBM-bandwidth-bound).
"""

import numpy as np
import ml_dtypes
from contextlib import ExitStack

import concourse.bass as bass
import concourse.tile as tile
from concourse import bacc, bass_isa, mybir
from concourse.bass_utils import run_bass_kernel_spmd

F32 = mybir.dt.float32
BF16 = mybir.dt.bfloat16
BF = ml_dtypes.bfloat16

B, S, D, F, E, R = 4, 2048, 1024, 4096, 8, 4
NCORES = 8
NT = B * S          # 8192 tokens total
N = NT // NCORES    # 1024 tokens per core
ER = E * R          # 32 lora rows
KD = D // 128       # 8 contraction tiles over D
KF = F // 128       # 32 contraction tiles over F
TH = N // 512       # 2 token halves (matmul moving dim)
P = 128

Relu = mybir.ActivationFunctionType.Relu


def _emit(ctx: ExitStack, tc: tile.TileContext, io: dict):
    nc = tc.nc

    consts = ctx.enter_context(tc.tile_pool(name="consts", bufs=1))
    xpool = ctx.enter_context(tc.tile_pool(name="xpool", bufs=1))
    wipool = ctx.enter_context(tc.tile_pool(name="wipool", bufs=1))
    ipool = ctx.enter_context(tc.tile_pool(name="ipool", bufs=1))
    wop = ctx.enter_context(tc.tile_pool(name="wop", bufs=2))
    rwork = ctx.enter_context(tc.tile_pool(name="rwork", bufs=2))
    rwork1 = ctx.enter_context(tc.tile_pool(name="rwork1", bufs=1))
    outp = ctx.enter_context(tc.tile_pool(name="outp", bufs=2))
    sps = ctx.enter_context(tc.tile_pool(name="sps", bufs=1, space="PSUM"))
    bps = ctx.enter_context(tc.tile_pool(name="bps", bufs=6, space="PSUM"))

    # ---------- PE warm-up first: one long accumulation group (no PSUM
    # drain semaphores between matmuls) so the PE runs back-to-back from
    # engine-boot and climbs its p-state ramp while the input DMAs land;
    # it should still be spinning when x16 arrives so the router starts
    # at full clock with no idle gap (idle >100ns resets the ramp).
    NWARM = 23
    inter0 = ipool.tile([P, N], BF16, tag="inter0", name="inter0")
    warm_src = inter0[:, 0:512]
    nc.vector.memset(warm_src, 1.0)
    psum_w = bps.tile([P, 512], F32, tag="pbig", name="pwarm")
    for w in range(NWARM):
        nc.tensor.matmul(psum_w, lhsT=warm_src[:, 0:P], rhs=warm_src,
                         start=(w == 0), stop=(w == NWARM - 1))

    # ---------- constants ----------
    # rep[e, e*R + r] = 1  (bf16): expands the [8, t] expert mask to the
    # [32, t] lora-row mask via one tiny matmul per token-half
    rep = consts.tile([E, ER], BF16, tag="rep")
    nc.gpsimd.dma_start(out=rep, in_=io["repM"])
    # biases [128, 49] f32: cols 0:32 wi_b by f-tile, 32:40 wo_b by d-tile,
    # 40:48 gate_b replicated (unused), 48 gate_b transposed (partition e)
    biases_sb = consts.tile([P, 49], F32, tag="biases")
    nc.gpsimd.dma_start(out=biases_sb, in_=io["biases"])
    wib_sb = biases_sb[:, 0:KF]
    wob_sb = biases_sb[:, KF:KF + KD]
    gateb_col = biases_sb[:, 48:49]
    # cg [D, 72] bf16: cols 0:32 Acat.T, 32:40 g16, 40:64 zeros, 64:72 dg16
    # (zero block keeps the dlogit psum rows 32-aligned). Running the same
    # stationary over both dx16 and x16 makes rows 32:40 = x@g16 and rows
    # 64:72 = x@dg16, so logits = rows 32:40 + rows 64:72 is fp32-accurate
    # (keeps reference tie-breaks) and rows 0:32 give fp32-accurate u.
    CW = 72
    cg_big = consts.tile([P, KD * CW], BF16, tag="cg")
    nc.gpsimd.dma_start(out=cg_big, in_=io["cgT"])
    cg_sb = [cg_big[:, k * CW:(k + 1) * CW] for k in range(KD)]
    # bcat packed [64, F/2]: rows 32q:32q+32 hold Bcat.T columns q*2048:(q+1)*2048
    # (halves the per-partition SBUF cost; matmul bases stay in {0, 32})
    bcat_sb = consts.tile([64, F // 2], BF16, tag="bc")
    nc.gpsimd.dma_start(out=bcat_sb, in_=io["bT"])

    def bcat_lhsT(f):
        q, fr = divmod(f, 16)
        return bcat_sb[32 * q:32 * (q + 1), fr * P:(fr + 1) * P]

    # ---------- resident activations / weights ----------
    inter_sb = [inter0] + [
        ipool.tile([P, N], BF16, tag=f"inter{f}", name=f"inter{f}")
        for f in range(1, KF)]
    # masked-u replicated into both 32-partition groups (rows 0:32 and 32:64
    # hold u_m) so each packed-bcat chunk finds its fmap at its base partition
    um2 = [consts.tile([64, 512], BF16, tag=f"um{th}", name=f"um{th}")
           for th in range(TH)]

    # ---------- DMA priority order on the sync queue (bandwidth-bound, so
    # order = arrival time): dx16 -> x16 halves -> wi eighths -> wo.
    # All input tensors are pre-tiled on the host to the exact SBUF layout,
    # so every DMA is a plain contiguous copy (16KB/partition descriptors
    # instead of 2KB gather lines).
    QF = F // 8   # 512 f-columns per wi eighth
    HK = KD // 2
    # dx16 first: the dx correction pass is the PE's gap-filler while the
    # x16 halves are still on the wire
    dx16 = xpool.tile([P, KD * N], BF16, tag="dx16")
    nc.sync.dma_start(out=dx16, in_=io["dxT16"])
    x16h = []
    for h in range(2):
        t = xpool.tile([P, HK * N], BF16, tag=f"x16h{h}", name=f"x16h{h}")
        nc.sync.dma_start(out=t, in_=io["xT16"][h])
        x16h.append(t)
    x16 = [x16h[k // HK][:, (k % HK) * N:(k % HK + 1) * N] for k in range(KD)]
    wi_q = []
    for q in range(8):
        wq = wipool.tile([P, KD * QF], BF16, tag=f"wiq{q}", name=f"wiq{q}")
        nc.sync.dma_start(out=wq, in_=io["wiT"][q])
        wi_q.append(wq)

    def wi_lhsT(k, f):
        q, fr = divmod(f, 4)
        return wi_q[q][:, k * QF + fr * P:k * QF + (fr + 1) * P]

    # ---------- router + lora projection, one fused group ----------
    # [u | logits | dlogits] = (x16 + dx16) @ [Acat | g16 | dg16]:
    # running the same stationary over both x16 and dx16 makes rows 32:40
    # equal x@g16 and rows 64:72 equal x@dg16 (both fp32-accurate), so
    # logits = rows 32:40 + rows 64:72 keeps reference tie-breaks.
    TS = [slice(th * 512, (th + 1) * 512) for th in range(TH)]
    psum_cu = [sps.tile([CW, 512], F32, tag=f"pcu{th}", name=f"pcu{th}")
               for th in range(TH)]
    # dx pass first (dx16 arrives first), x pass second closes the group
    for k in range(KD):
        for th in range(TH):
            nc.tensor.matmul(psum_cu[th], lhsT=cg_sb[k],
                             rhs=dx16[:, k * N:k * N + N][:, TS[th]],
                             start=(k == 0), stop=False)
    for k in range(KD):
        for th in range(TH):
            nc.tensor.matmul(psum_cu[th], lhsT=cg_sb[k],
                             rhs=x16[k][:, TS[th]],
                             start=False, stop=(k == KD - 1))

    # mm1 group helpers. Early groups are emitted th-split (one psum bank
    # each) so up to 6 x-term groups can stay open while the mask chain
    # (DVE/GPSIMD, ~8us) runs — the PE keeps crunching x-terms instead of
    # idling for um; later groups pair th0/th1 per stationary load.
    mm1_ps = {}

    def mm1_x_terms(f, ths):
        for th in ths:
            ps = bps.tile([P, 512], F32, tag="pbig", name=f"p1_{f}_{th}")
            mm1_ps[(f, th)] = ps
            for k in range(KD):
                nc.tensor.matmul(ps, lhsT=wi_lhsT(k, f),
                                 rhs=x16[k][:, TS[th]],
                                 start=(k == 0), stop=False)

    def mm1_x_terms_paired(f):
        ps = [bps.tile([P, 512], F32, tag="pbig", name=f"p1_{f}_{th}")
              for th in range(TH)]
        for th in range(TH):
            mm1_ps[(f, th)] = ps[th]
        for k in range(KD):
            for th in range(TH):
                nc.tensor.matmul(ps[th], lhsT=wi_lhsT(k, f),
                                 rhs=x16[k][:, TS[th]],
                                 start=(k == 0), stop=False)

    def mm1_finish(f, ths=range(TH)):
        q = f // 16
        for th in ths:
            ps = mm1_ps.pop((f, th))
            nc.tensor.matmul(ps, lhsT=bcat_lhsT(f),
                             rhs=um2[th][32 * q:32 * (q + 1), :],
                             start=False, stop=True)
            # activations split across the scalar and vector engines so
            # psum banks recycle at twice the single-engine drain rate
            if th == 0:
                nc.scalar.activation(inter_sb[f][:, TS[0]], ps, Relu,
                                     bias=wib_sb[:, f:f + 1])
            else:
                nc.vector.tensor_scalar(inter_sb[f][:, TS[1]], ps,
                                        wib_sb[:, f:f + 1], 0.0,
                                        mybir.AluOpType.add,
                                        mybir.AluOpType.max)

    # six single-bank x-term groups keep the PE busy through the mask chain
    mm1_x_terms(0, [0, 1])
    mm1_x_terms(1, [0, 1])
    mm1_x_terms(2, [0, 1])

    # ---------- top-1 mask, row-parallel (no transposes) ----------
    # u is copied to SBUF on the scalar engine, which both sidesteps the
    # one-PSUM-operand DVE limit and frees the router psum banks for the
    # mask-replication matmuls below
    u_sb = []
    lgs = []
    for th in range(TH):
        usb = rwork.tile([ER, 512], BF16, tag="usb", name=f"usb{th}")
        nc.scalar.activation(usb, psum_cu[th][0:ER, :],
                             mybir.ActivationFunctionType.Copy)
        u_sb.append(usb)
        lga = rwork.tile([E, 512], F32, tag="lga", name=f"lga{th}")
        nc.vector.tensor_scalar_add(lga, psum_cu[th][ER:ER + E, :],
                                    gateb_col[0:E, :])
        lg = rwork.tile([E, 512], F32, tag="lg", name=f"lg{th}")
        nc.vector.tensor_add(lg, psum_cu[th][64:CW, :], lga)
        lgs.append(lg)

    # ---------- masked lora projection ----------
    # maskT32 = rep.T @ mask8 (tiny matmul), um = u * maskT32
    for th in range(TH):
        mx = rwork.tile([E, 512], F32, tag="mx", name=f"mx{th}")
        nc.gpsimd.partition_all_reduce(mx, lgs[th], channels=E,
                                       reduce_op=bass_isa.ReduceOp.max)
        m8 = rwork.tile([E, 512], BF16, tag="m8", name=f"m8{th}")
        nc.vector.tensor_tensor(m8, lgs[th], mx, mybir.AluOpType.is_equal)
        psum_m = sps.tile([ER, 512], F32, tag=f"pcu{th}", name=f"pm{th}")
        nc.tensor.matmul(psum_m, lhsT=rep, rhs=m8,
                         start=True, stop=True)
        for q in range(2):
            nc.vector.tensor_mul(um2[th][32 * q:32 * (q + 1), :],
                                 psum_m, u_sb[th])

    # ---------- matmul 1: interT = relu(wi @ x.T + Bcat.T @ u_m + wi_b) ------
    for f in range(3):
        mm1_finish(f)
    for f in range(3, KF):
        mm1_x_terms_paired(f)
        mm1_finish(f)

    # ---------- matmul 2: outT = wo @ inter + wo_b ----------
    # woT [F, D] column-block d fetched as ONE 3D DMA into [p, (kf j)] layout:
    # wo_big[p, kf*128 + j] = woT[kf*128 + p, d*128 + j]
    for d in range(KD):
        wo_big = wop.tile([P, F], BF16, tag="wo", name=f"wo{d}")
        nc.sync.dma_start(out=wo_big, in_=io["woTt"][d])
        ps = [bps.tile([P, 512], F32, tag="pbig", name=f"p2_{d}_{th}")
              for th in range(TH)]
        for kf in range(KF):
            for th in range(TH):
                nc.tensor.matmul(ps[th], lhsT=wo_big[:, kf * P:(kf + 1) * P],
                                 rhs=inter_sb[kf][:, TS[th]],
                                 start=(kf == 0), stop=(kf == KF - 1))
        osb = outp.tile([P, N], F32, tag="osb")
        if d < KD - 1:
            for th in range(TH):
                nc.vector.tensor_scalar(osb[:, TS[th]], ps[th],
                                        wob_sb[:, d:d + 1], None,
                                        mybir.AluOpType.add)
            nc.gpsimd.dma_start(out=io["outT"][d * P:(d + 1) * P, :], in_=osb)
        else:
            # last d-tile: store each half as soon as its psum closes so the
            # th0 transfer overlaps the final matmuls and the tail transfer
            # is half-sized
            for th in range(TH):
                nc.vector.tensor_scalar(osb[:, TS[th]], ps[th],
                                        wob_sb[:, d:d + 1], None,
                                        mybir.AluOpType.add)
                nc.gpsimd.dma_start(out=io["outT"][d * P:(d + 1) * P, TS[th]],
                                    in_=osb[:, TS[th]])


_CACHED_NC = None


def build_nc():
    global _CACHED_NC
    if _CACHED_NC is not None:
        return _CACHED_NC
    nc = bacc.Bacc("TRN2", target_bir_lowering=False, debug=False,
                   enable_asserts=False, num_devices=NCORES)
    decls = [
        ("xT16", [2, P, (KD // 2) * N], BF16, False),
        ("dxT16", [P, KD * N], BF16, False),
        ("cgT", [P, KD * 72], BF16, False),
        ("repM", [E, ER], BF16, False),
        ("biases", [P, 49], F32, False),
        ("bT", [64, F // 2], BF16, False),
        ("wiT", [8, P, KD * (F // 8)], BF16, False),
        ("woTt", [KD, P, F], BF16, False),
        ("outT", [D, N], F32, True),
    ]
    io = {}
    for name, shape, dt_, is_out in decls:
        io[name] = nc.dram_tensor(
            name, shape, dt_, kind="ExternalOutput" if is_out else "ExternalInput"
        ).ap()
    with tile.TileContext(nc) as tc:
        with ExitStack() as ctx:
            _emit(ctx, tc, io)
    nc.compile()
    _CACHED_NC = nc
    return nc


def make_in_maps(inputs: dict) -> list[dict]:
    f32 = np.float32
    x = np.ascontiguousarray(np.asarray(inputs["hidden_states"], f32).reshape(NT, D))
    gT = np.asarray(inputs["gate_W"], f32).T                                # [D, E]
    g16 = gT.astype(BF)
    dg16 = (gT - g16.astype(f32)).astype(BF)
    aT = np.asarray(inputs["lora_A"], f32).reshape(ER, D).T                 # [D, 32]
    cgT_flat = np.concatenate(
        [aT.astype(BF), g16, np.zeros((D, 24), BF), dg16], axis=1)          # [D, 72]
    cgT = np.ascontiguousarray(
        cgT_flat.reshape(KD, P, 72).transpose(1, 0, 2).reshape(P, KD * 72))
    biases = np.zeros((P, 49), f32)
    biases[:, 0:KF] = np.asarray(inputs["wi_b"], f32).reshape(KF, P).T
    biases[:, KF:KF + KD] = np.asarray(inputs["wo_b"], f32).reshape(KD, P).T
    biases[:, KF + KD:KF + KD + E] = np.asarray(inputs["gate_b"], f32)[None, :]
    biases[0:E, 48] = np.asarray(inputs["gate_b"], f32)
    bT_flat = np.asarray(inputs["lora_B"], f32).transpose(0, 2, 1).reshape(ER, F)
    # packed [64, F/2]: rows 32q:32q+32 = Bcat.T columns q*2048:(q+1)*2048
    bT = np.ascontiguousarray(
        bT_flat.reshape(ER, 2, F // 2).transpose(1, 0, 2).reshape(64, F // 2).astype(BF))
    # wi pre-tiled per eighth: wiT[q, p, k*512 + f] = wi_W.T[k*128+p, q*512+f]
    wiT = np.ascontiguousarray(
        np.asarray(inputs["wi_W"], f32).T.astype(BF)
        .reshape(KD, P, 8, F // 8).transpose(2, 1, 0, 3).reshape(8, P, KD * (F // 8)))
    woT = np.asarray(inputs["wo_W"], f32).T.astype(BF)                      # [F, D]
    # pre-tiled to SBUF layout: woTt[d, p, kf*128+j] = woT[kf*128+p, d*128+j]
    woTt = np.ascontiguousarray(
        woT.reshape(KF, P, KD, P).transpose(2, 1, 0, 3).reshape(KD, P, F))
    repM = np.ascontiguousarray(
        (np.arange(E)[:, None] == (np.arange(ER)[None, :] // R)).astype(BF))

    in_maps = []
    for c in range(NCORES):
        xT32 = x[c * N:(c + 1) * N].T                                       # [D, N]
        xT16 = xT32.astype(BF)
        dxT16 = (xT32 - xT16.astype(f32)).astype(BF)
        # pre-tiled: xT16[h, p, k4*1024 + t] = x.T[(h*4+k4)*128 + p, t]
        xT16t = np.ascontiguousarray(
            xT16.reshape(2, KD // 2, P, N).transpose(0, 2, 1, 3)
            .reshape(2, P, (KD // 2) * N))
        dxT16t = np.ascontiguousarray(
            dxT16.reshape(KD, P, N).transpose(1, 0, 2).reshape(P, KD * N))
        in_maps.append({
            "xT16": xT16t,
            "dxT16": dxT16t,
            "cgT": cgT, "repM": repM, "biases": biases, "bT": bT,
            "wiT": wiT, "woTt": woTt,
        })
    return in_maps


def kernel(**inputs) -> np.ndarray:
    nc = build_nc()
    in_maps = make_in_maps(inputs)
    res = run_bass_kernel_spmd(nc, in_maps, core_ids=list(range(NCORES)))
    out = np.empty((NT, D), np.float32)
    for c in range(NCORES):
        out[c * N:(c + 1) * N] = res.results[c]["outT"].T
    return out.reshape(B, S, D)


# revision 75
# speedup vs baseline: 1.0072x; 1.0072x over previous
"""MoE block (top-1 routing, shared FFN + per-expert LoRA) on 8 TRN2 NeuronCores.

Strategy: data-parallel over the 8192 tokens (1024 tokens/core), weights
replicated. The reference's dense-then-mask expert loop collapses to:

    logits = x @ gate_W.T + gate_b ; e* = argmax(logits)
    u      = x @ A_cat.T                 [N, 32]
    u_m    = u * onehot-mask(e*)  (zero all but selected expert's 4 lora rows)
    inter  = relu(x @ wi_W.T + u_m @ B_cat + wi_b)
    out    = inter @ wo_W.T + wo_b

Everything runs in transposed (feature-major) layout on chip; the host
pre-transposes the shards/weights and re-transposes the output.

Router logits/u are fp32-accurate via a split-bf16 scheme (x16+dx16 against
[Acat|g16|dg16]) so routing tie-breaks match the reference. The top-1 mask is
built without PE/DVE transpose ping-pong: logits stay expert-major [8, 512],
a GPSIMD partition_all_reduce gives the per-token max on all partitions,
is_equal gives the mask, and a tiny [8->32] replication matmul expands it to
the lora rows. All input tensors are pre-tiled on the host so every DMA is a
contiguous copy (the input stream is HBM-bandwidth-bound).
"""

import numpy as np
import ml_dtypes
from contextlib import ExitStack

import concourse.bass as bass
import concourse.tile as tile
from concourse import bacc, bass_isa, mybir
from concourse.bass_utils import run_bass_kernel_spmd

F32 = mybir.dt.float32
BF16 = mybir.dt.bfloat16
BF = ml_dtypes.bfloat16

B, S, D, F, E, R = 4, 2048, 1024, 4096, 8, 4
NCORES = 8
NT = B * S          # 8192 tokens total
N = NT // NCORES    # 1024 tokens per core
ER = E * R          # 32 lora rows
KD = D // 128       # 8 contraction tiles over D
KF = F // 128       # 32 contraction tiles over F
TH = N // 512       # 2 token halves (matmul moving dim)
P = 128

Relu = mybir.ActivationFunctionType.Relu


def _emit(ctx: ExitStack, tc: tile.TileContext, io: dict):
    nc = tc.nc

    consts = ctx.enter_context(tc.tile_pool(name="consts", bufs=1))
    xpool = ctx.enter_context(tc.tile_pool(name="xpool", bufs=1))
    wipool = ctx.enter_context(tc.tile_pool(name="wipool", bufs=1))
    ipool = ctx.enter_context(tc.tile_pool(name="ipool", bufs=1))
    wop = ctx.enter_context(tc.tile_pool(name="wop", bufs=2))
    rwork = ctx.enter_context(tc.tile_pool(name="rwork", bufs=2))
    rwork1 = ctx.enter_context(tc.tile_pool(name="rwork1", bufs=1))
    outp = ctx.enter_context(tc.tile_pool(name="outp", bufs=2))
    sps = ctx.enter_context(tc.tile_pool(name="sps", bufs=1, space="PSUM"))
    bps = ctx.enter_context(tc.tile_pool(name="bps", bufs=6, space="PSUM"))

    # ---------- PE warm-up first: one long accumulation group (no PSUM
    # drain semaphores between matmuls) so the PE runs back-to-back from
    # engine-boot and climbs its p-state ramp while the input DMAs land;
    # it should still be spinning when x16 arrives so the router starts
    # at full clock with no idle gap (idle >100ns resets the ramp).
    NWARM = 23
    inter0 = ipool.tile([P, N], BF16, tag="inter0", name="inter0")
    warm_src = inter0[:, 0:512]
    nc.vector.memset(warm_src, 1.0)
    psum_w = bps.tile([P, 512], F32, tag="pbig", name="pwarm")
    for w in range(NWARM):
        nc.tensor.matmul(psum_w, lhsT=warm_src[:, 0:P], rhs=warm_src,
                         start=(w == 0), stop=(w == NWARM - 1))

    # ---------- constants ----------
    # rep[e, e*R + r] = 1  (bf16): expands the [8, t] expert mask to the
    # [32, t] lora-row mask via one tiny matmul per token-half
    rep = consts.tile([E, ER], BF16, tag="rep")
    nc.gpsimd.dma_start(out=rep, in_=io["repM"])
    # biases [128, 49] f32: cols 0:32 wi_b by f-tile, 32:40 wo_b by d-tile,
    # 40:48 gate_b replicated (unused), 48 gate_b transposed (partition e)
    biases_sb = consts.tile([P, 49], F32, tag="biases")
    nc.gpsimd.dma_start(out=biases_sb, in_=io["biases"])
    wib_sb = biases_sb[:, 0:KF]
    wob_sb = biases_sb[:, KF:KF + KD]
    gateb_col = biases_sb[:, 48:49]
    # cg [D, 72] bf16: cols 0:32 Acat.T, 32:40 g16, 40:64 zeros, 64:72 dg16
    # (zero block keeps the dlogit psum rows 32-aligned). Running the same
    # stationary over both dx16 and x16 makes rows 32:40 = x@g16 and rows
    # 64:72 = x@dg16, so logits = rows 32:40 + rows 64:72 is fp32-accurate
    # (keeps reference tie-breaks) and rows 0:32 give fp32-accurate u.
    CW = 72
    cg_big = consts.tile([P, KD * CW], BF16, tag="cg")
    nc.gpsimd.dma_start(out=cg_big, in_=io["cgT"])
    cg_sb = [cg_big[:, k * CW:(k + 1) * CW] for k in range(KD)]
    # bcat packed [64, F/2]: rows 32q:32q+32 hold Bcat.T columns q*2048:(q+1)*2048
    # (halves the per-partition SBUF cost; matmul bases stay in {0, 32})
    bcat_sb = consts.tile([64, F // 2], BF16, tag="bc")
    nc.gpsimd.dma_start(out=bcat_sb, in_=io["bT"])

    def bcat_lhsT(f):
        q, fr = divmod(f, 16)
        return bcat_sb[32 * q:32 * (q + 1), fr * P:(fr + 1) * P]

    # ---------- resident activations / weights ----------
    inter_sb = [inter0] + [
        ipool.tile([P, N], BF16, tag=f"inter{f}", name=f"inter{f}")
        for f in range(1, KF)]
    # masked-u replicated into both 32-partition groups (rows 0:32 and 32:64
    # hold u_m) so each packed-bcat chunk finds its fmap at its base partition
    um2 = [consts.tile([64, 512], BF16, tag=f"um{th}", name=f"um{th}")
           for th in range(TH)]

    # ---------- DMA priority order on the sync queue (bandwidth-bound, so
    # order = arrival time): dx16 -> x16 halves -> wi eighths -> wo.
    # All input tensors are pre-tiled on the host to the exact SBUF layout,
    # so every DMA is a plain contiguous copy (16KB/partition descriptors
    # instead of 2KB gather lines).
    QF = F // 8   # 512 f-columns per wi eighth
    HK = KD // 2
    # dx16 first: the dx correction pass is the PE's gap-filler while the
    # x16 halves are still on the wire
    dx16 = xpool.tile([P, KD * N], BF16, tag="dx16")
    nc.sync.dma_start(out=dx16, in_=io["dxT16"])
    x16h = []
    for h in range(2):
        t = xpool.tile([P, HK * N], BF16, tag=f"x16h{h}", name=f"x16h{h}")
        nc.sync.dma_start(out=t, in_=io["xT16"][h])
        x16h.append(t)
    x16 = [x16h[k // HK][:, (k % HK) * N:(k % HK + 1) * N] for k in range(KD)]
    wi_q = []
    for q in range(8):
        wq = wipool.tile([P, KD * QF], BF16, tag=f"wiq{q}", name=f"wiq{q}")
        nc.sync.dma_start(out=wq, in_=io["wiT"][q])
        wi_q.append(wq)

    def wi_lhsT(k, f):
        q, fr = divmod(f, 4)
        return wi_q[q][:, k * QF + fr * P:k * QF + (fr + 1) * P]

    # ---------- router + lora projection, one fused group ----------
    # [u | logits | dlogits] = (x16 + dx16) @ [Acat | g16 | dg16]:
    # running the same stationary over both x16 and dx16 makes rows 32:40
    # equal x@g16 and rows 64:72 equal x@dg16 (both fp32-accurate), so
    # logits = rows 32:40 + rows 64:72 keeps reference tie-breaks.
    TS = [slice(th * 512, (th + 1) * 512) for th in range(TH)]
    psum_cu = [sps.tile([CW, 512], F32, tag=f"pcu{th}", name=f"pcu{th}")
               for th in range(TH)]
    # dx pass first (dx16 arrives first), x pass second closes the group
    for k in range(KD):
        for th in range(TH):
            nc.tensor.matmul(psum_cu[th], lhsT=cg_sb[k],
                             rhs=dx16[:, k * N:k * N + N][:, TS[th]],
                             start=(k == 0), stop=False)
    for k in range(KD):
        for th in range(TH):
            nc.tensor.matmul(psum_cu[th], lhsT=cg_sb[k],
                             rhs=x16[k][:, TS[th]],
                             start=False, stop=(k == KD - 1))

    # mm1 group helpers. Early groups are emitted th-split (one psum bank
    # each) so up to 6 x-term groups can stay open while the mask chain
    # (DVE/GPSIMD, ~8us) runs — the PE keeps crunching x-terms instead of
    # idling for um; later groups pair th0/th1 per stationary load.
    mm1_ps = {}

    def mm1_x_terms(f, ths):
        for th in ths:
            ps = bps.tile([P, 512], F32, tag="pbig", name=f"p1_{f}_{th}")
            mm1_ps[(f, th)] = ps
            for k in range(KD):
                nc.tensor.matmul(ps, lhsT=wi_lhsT(k, f),
                                 rhs=x16[k][:, TS[th]],
                                 start=(k == 0), stop=False)

    def mm1_x_terms_paired(f):
        ps = [bps.tile([P, 512], F32, tag="pbig", name=f"p1_{f}_{th}")
              for th in range(TH)]
        for th in range(TH):
            mm1_ps[(f, th)] = ps[th]
        for k in range(KD):
            for th in range(TH):
                nc.tensor.matmul(ps[th], lhsT=wi_lhsT(k, f),
                                 rhs=x16[k][:, TS[th]],
                                 start=(k == 0), stop=False)

    def mm1_finish(f, ths=range(TH)):
        q = f // 16
        for th in ths:
            ps = mm1_ps.pop((f, th))
            nc.tensor.matmul(ps, lhsT=bcat_lhsT(f),
                             rhs=um2[th][32 * q:32 * (q + 1), :],
                             start=False, stop=True)
            # activations split across the scalar and vector engines so
            # psum banks recycle at twice the single-engine drain rate
            if th == 0:
                nc.scalar.activation(inter_sb[f][:, TS[0]], ps, Relu,
                                     bias=wib_sb[:, f:f + 1])
            else:
                nc.vector.tensor_scalar(inter_sb[f][:, TS[1]], ps,
                                        wib_sb[:, f:f + 1], 0.0,
                                        mybir.AluOpType.add,
                                        mybir.AluOpType.max)

    # six single-bank x-term groups keep the PE busy through the mask chain
    mm1_x_terms(0, [0, 1])
    mm1_x_terms(1, [0, 1])
    mm1_x_terms(2, [0, 1])

    # ---------- top-1 mask, row-parallel (no transposes) ----------
    # u is copied to SBUF on the scalar engine, which both sidesteps the
    # one-PSUM-operand DVE limit and frees the router psum banks for the
    # mask-replication matmuls below
    u_sb = []
    lgs = []
    for th in range(TH):
        usb = rwork.tile([ER, 512], BF16, tag="usb", name=f"usb{th}")
        nc.scalar.activation(usb, psum_cu[th][0:ER, :],
                             mybir.ActivationFunctionType.Copy)
        u_sb.append(usb)
        lga = rwork.tile([E, 512], F32, tag="lga", name=f"lga{th}")
        nc.vector.tensor_scalar_add(lga, psum_cu[th][ER:ER + E, :],
                                    gateb_col[0:E, :])
        lg = rwork.tile([E, 512], F32, tag="lg", name=f"lg{th}")
        nc.vector.tensor_add(lg, psum_cu[th][64:CW, :], lga)
        lgs.append(lg)

    # ---------- masked lora projection ----------
    # maskT32 = rep.T @ mask8 (tiny matmul), um = u * maskT32
    for th in range(TH):
        mx = rwork.tile([E, 512], F32, tag="mx", name=f"mx{th}")
        nc.gpsimd.partition_all_reduce(mx, lgs[th], channels=E,
                                       reduce_op=bass_isa.ReduceOp.max)
        m8 = rwork.tile([E, 512], BF16, tag="m8", name=f"m8{th}")
        nc.vector.tensor_tensor(m8, lgs[th], mx, mybir.AluOpType.is_equal)
        psum_m = sps.tile([ER, 512], F32, tag=f"pcu{th}", name=f"pm{th}")
        nc.tensor.matmul(psum_m, lhsT=rep, rhs=m8,
                         start=True, stop=True)
        for q in range(2):
            nc.vector.tensor_mul(um2[th][32 * q:32 * (q + 1), :],
                                 psum_m, u_sb[th])

    # ---------- matmul 1: interT = relu(wi @ x.T + Bcat.T @ u_m + wi_b) ------
    for f in range(3):
        mm1_finish(f)
    for f in range(3, KF):
        mm1_x_terms_paired(f)
        mm1_finish(f)

    # ---------- matmul 2: outT = wo @ inter + wo_b ----------
    # woT [F, D] column-block d fetched as ONE 3D DMA into [p, (kf j)] layout:
    # wo_big[p, kf*128 + j] = woT[kf*128 + p, d*128 + j]
    for d in range(KD):
        wo_big = wop.tile([P, F], BF16, tag="wo", name=f"wo{d}")
        nc.sync.dma_start(out=wo_big, in_=io["woTt"][d])
        ps = [bps.tile([P, 512], F32, tag="pbig", name=f"p2_{d}_{th}")
              for th in range(TH)]
        for kf in range(KF):
            for th in range(TH):
                nc.tensor.matmul(ps[th], lhsT=wo_big[:, kf * P:(kf + 1) * P],
                                 rhs=inter_sb[kf][:, TS[th]],
                                 start=(kf == 0), stop=(kf == KF - 1))
        osb = outp.tile([P, N], F32, tag="osb")
        if d < KD - 1:
            for th in range(TH):
                nc.vector.tensor_scalar(osb[:, TS[th]], ps[th],
                                        wob_sb[:, d:d + 1], None,
                                        mybir.AluOpType.add)
            nc.gpsimd.dma_start(out=io["outT"][d * P:(d + 1) * P, :], in_=osb)
        else:
            # last d-tile: store each half as soon as its psum closes so the
            # th0 transfer overlaps the final matmuls and the tail transfer
            # is half-sized
            for th in range(TH):
                nc.vector.tensor_scalar(osb[:, TS[th]], ps[th],
                                        wob_sb[:, d:d + 1], None,
                                        mybir.AluOpType.add)
                nc.gpsimd.dma_start(out=io["outT"][d * P:(d + 1) * P, TS[th]],
                                    in_=osb[:, TS[th]])


_CACHED_NC = None


def build_nc():
    global _CACHED_NC
    if _CACHED_NC is not None:
        return _CACHED_NC
    nc = bacc.Bacc("TRN2", target_bir_lowering=False, debug=False,
                   enable_asserts=False, num_devices=NCORES)
    decls = [
        ("xT16", [2, P, (KD // 2) * N], BF16, False),
        ("dxT16", [P, KD * N], BF16, False),
        ("cgT", [P, KD * 72], BF16, False),
        ("repM", [E, ER], BF16, False),
        ("biases", [P, 49], F32, False),
        ("bT", [64, F // 2], BF16, False),
        ("wiT", [8, P, KD * (F // 8)], BF16, False),
        ("woTt", [KD, P, F], BF16, False),
        ("outT", [D, N], F32, True),
    ]
    io = {}
    for name, shape, dt_, is_out in decls:
        io[name] = nc.dram_tensor(
            name, shape, dt_, kind="ExternalOutput" if is_out else "ExternalInput"
        ).ap()
    with tile.TileContext(nc) as tc:
        with ExitStack() as ctx:
            _emit(ctx, tc, io)
    nc.compile()
    _CACHED_NC = nc
    return nc


def make_in_maps(inputs: dict) -> list[dict]:
    f32 = np.float32
    x = np.ascontiguousarray(np.asarray(inputs["hidden_states"], f32).reshape(NT, D))
    gT = np.asarray(inputs["gate_W"], f32).T                                # [D, E]
    g16 = gT.astype(BF)
    dg16 = (gT - g16.astype(f32)).astype(BF)
    aT = np.asarray(inputs["lora_A"], f32).reshape(ER, D).T                 # [D, 32]
    cgT_flat = np.concatenate(
        [aT.astype(BF), g16, np.zeros((D, 24), BF), dg16], axis=1)          # [D, 72]
    cgT = np.ascontiguousarray(
        cgT_flat.reshape(KD, P, 72).transpose(1, 0, 2).reshape(P, KD * 72))
    biases = np.zeros((P, 49), f32)
    biases[:, 0:KF] = np.asarray(inputs["wi_b"], f32).reshape(KF, P).T
    biases[:, KF:KF + KD] = np.asarray(inputs["wo_b"], f32).reshape(KD, P).T
    biases[:, KF + KD:KF + KD + E] = np.asarray(inputs["gate_b"], f32)[None, :]
    biases[0:E, 48] = np.asarray(inputs["gate_b"], f32)
    bT_flat = np.asarray(inputs["lora_B"], f32).transpose(0, 2, 1).reshape(ER, F)
    # packed [64, F/2]: rows 32q:32q+32 = Bcat.T columns q*2048:(q+1)*2048
    bT = np.ascontiguousarray(
        bT_flat.reshape(ER, 2, F // 2).transpose(1, 0, 2).reshape(64, F // 2).astype(BF))
    # wi pre-tiled per eighth: wiT[q, p, k*512 + f] = wi_W.T[k*128+p, q*512+f]
    wiT = np.ascontiguousarray(
        np.asarray(inputs["wi_W"], f32).T.astype(BF)
        .reshape(KD, P, 8, F // 8).transpose(2, 1, 0, 3).reshape(8, P, KD * (F // 8)))
    woT = np.asarray(inputs["wo_W"], f32).T.astype(BF)                      # [F, D]
    # pre-tiled to SBUF layout: woTt[d, p, kf*128+j] = woT[kf*128+p, d*128+j]
    woTt = np.ascontiguousarray(
        woT.reshape(KF, P, KD, P).transpose(2, 1, 0, 3).reshape(KD, P, F))
    repM = np.ascontiguousarray(
        (np.arange(E)[:, None] == (np.arange(ER)[None, :] // R)).astype(BF))

    in_maps = []
    for c in range(NCORES):
        xT32 = x[c * N:(c + 1) * N].T                                       # [D, N]
        xT16 = xT32.astype(BF)
        dxT16 = (xT32 - xT16.astype(f32)).astype(BF)
        # pre-tiled: xT16[h, p, k4*1024 + t] = x.T[(h*4+k4)*128 + p, t]
        xT16t = np.ascontiguousarray(
            xT16.reshape(2, KD // 2, P, N).transpose(0, 2, 1, 3)
            .reshape(2, P, (KD // 2) * N))
        dxT16t = np.ascontiguousarray(
            dxT16.reshape(KD, P, N).transpose(1, 0, 2).reshape(P, KD * N))
        in_maps.append({
            "xT16": xT16t,
            "dxT16": dxT16t,
            "cgT": cgT, "repM": repM, "biases": biases, "bT": bT,
            "wiT": wiT, "woTt": woTt,
        })
    return in_maps


def kernel(**inputs) -> np.ndarray:
    nc = build_nc()
    in_maps = make_in_maps(inputs)
    res = run_bass_kernel_spmd(nc, in_maps, core_ids=list(range(NCORES)))
    out = np.empty((NT, D), np.float32)
    for c in range(NCORES):
        out[c * N:(c + 1) * N] = res.results[c]["outT"].T
    return out.reshape(B, S, D)


# revision 76
# speedup vs baseline: 1.0109x; 1.0037x over previous
"""MoE block (top-1 routing, shared FFN + per-expert LoRA) on 8 TRN2 NeuronCores.

Strategy: data-parallel over the 8192 tokens (1024 tokens/core), weights
replicated. The reference's dense-then-mask expert loop collapses to:

    logits = x @ gate_W.T + gate_b ; e* = argmax(logits)
    u      = x @ A_cat.T                 [N, 32]
    u_m    = u * onehot-mask(e*)  (zero all but selected expert's 4 lora rows)
    inter  = relu(x @ wi_W.T + u_m @ B_cat + wi_b)
    out    = inter @ wo_W.T + wo_b

Everything runs in transposed (feature-major) layout on chip; the host
pre-transposes the shards/weights and re-transposes the output.

Router logits/u are fp32-accurate via a split-bf16 scheme (x16+dx16 against
[Acat|g16|dg16]) so routing tie-breaks match the reference. The top-1 mask is
built without PE/DVE transpose ping-pong: logits stay expert-major [8, 512],
a GPSIMD partition_all_reduce gives the per-token max on all partitions,
is_equal gives the mask, and a tiny [8->32] replication matmul expands it to
the lora rows. All input tensors are pre-tiled on the host so every DMA is a
contiguous copy (the input stream is HBM-bandwidth-bound).
"""

import numpy as np
import ml_dtypes
from contextlib import ExitStack

import concourse.bass as bass
import concourse.tile as tile
from concourse import bacc, bass_isa, mybir
from concourse.bass_utils import run_bass_kernel_spmd

F32 = mybir.dt.float32
BF16 = mybir.dt.bfloat16
BF = ml_dtypes.bfloat16

B, S, D, F, E, R = 4, 2048, 1024, 4096, 8, 4
NCORES = 8
NT = B * S          # 8192 tokens total
N = NT // NCORES    # 1024 tokens per core
ER = E * R          # 32 lora rows
KD = D // 128       # 8 contraction tiles over D
KF = F // 128       # 32 contraction tiles over F
TH = N // 512       # 2 token halves (matmul moving dim)
P = 128

Relu = mybir.ActivationFunctionType.Relu


def _emit(ctx: ExitStack, tc: tile.TileContext, io: dict):
    nc = tc.nc

    consts = ctx.enter_context(tc.tile_pool(name="consts", bufs=1))
    xpool = ctx.enter_context(tc.tile_pool(name="xpool", bufs=1))
    wipool = ctx.enter_context(tc.tile_pool(name="wipool", bufs=1))
    ipool = ctx.enter_context(tc.tile_pool(name="ipool", bufs=1))
    wop = ctx.enter_context(tc.tile_pool(name="wop", bufs=2))
    rwork = ctx.enter_context(tc.tile_pool(name="rwork", bufs=2))
    rwork1 = ctx.enter_context(tc.tile_pool(name="rwork1", bufs=1))
    outp = ctx.enter_context(tc.tile_pool(name="outp", bufs=2))
    sps = ctx.enter_context(tc.tile_pool(name="sps", bufs=1, space="PSUM"))
    bps = ctx.enter_context(tc.tile_pool(name="bps", bufs=6, space="PSUM"))

    # ---------- PE warm-up first: one long accumulation group (no PSUM
    # drain semaphores between matmuls) so the PE runs back-to-back from
    # engine-boot and climbs its p-state ramp while the input DMAs land;
    # it should still be spinning when x16 arrives so the router starts
    # at full clock with no idle gap (idle >100ns resets the ramp).
    NWARM = 23
    inter0 = ipool.tile([P, N], BF16, tag="inter0", name="inter0")
    warm_src = inter0[:, 0:512]
    nc.vector.memset(warm_src, 1.0)
    psum_w = bps.tile([P, 512], F32, tag="pbig", name="pwarm")
    for w in range(NWARM):
        nc.tensor.matmul(psum_w, lhsT=warm_src[:, 0:P], rhs=warm_src,
                         start=(w == 0), stop=(w == NWARM - 1))

    # ---------- constants ----------
    # rep[e, e*R + r] = 1  (bf16): expands the [8, t] expert mask to the
    # [32, t] lora-row mask via one tiny matmul per token-half
    rep = consts.tile([E, ER], BF16, tag="rep")
    nc.gpsimd.dma_start(out=rep, in_=io["repM"])
    # biases [128, 49] f32: cols 0:32 wi_b by f-tile, 32:40 wo_b by d-tile,
    # 40:48 gate_b replicated (unused), 48 gate_b transposed (partition e)
    biases_sb = consts.tile([P, 49], F32, tag="biases")
    nc.gpsimd.dma_start(out=biases_sb, in_=io["biases"])
    wib_sb = biases_sb[:, 0:KF]
    wob_sb = biases_sb[:, KF:KF + KD]
    gateb_col = biases_sb[:, 48:49]
    # cg [D, 72] bf16: cols 0:32 Acat.T, 32:40 g16, 40:64 zeros, 64:72 dg16
    # (zero block keeps the dlogit psum rows 32-aligned). Running the same
    # stationary over both dx16 and x16 makes rows 32:40 = x@g16 and rows
    # 64:72 = x@dg16, so logits = rows 32:40 + rows 64:72 is fp32-accurate
    # (keeps reference tie-breaks) and rows 0:32 give fp32-accurate u.
    CW = 72
    cg_big = consts.tile([P, KD * CW], BF16, tag="cg")
    nc.gpsimd.dma_start(out=cg_big, in_=io["cgT"])
    cg_sb = [cg_big[:, k * CW:(k + 1) * CW] for k in range(KD)]
    # bcat packed [64, F/2]: rows 32q:32q+32 hold Bcat.T columns q*2048:(q+1)*2048
    # (halves the per-partition SBUF cost; matmul bases stay in {0, 32})
    bcat_sb = consts.tile([64, F // 2], BF16, tag="bc")
    nc.gpsimd.dma_start(out=bcat_sb, in_=io["bT"])

    def bcat_lhsT(f):
        q, fr = divmod(f, 16)
        return bcat_sb[32 * q:32 * (q + 1), fr * P:(fr + 1) * P]

    # ---------- resident activations / weights ----------
    inter_sb = [inter0] + [
        ipool.tile([P, N], BF16, tag=f"inter{f}", name=f"inter{f}")
        for f in range(1, KF)]
    # masked-u replicated into both 32-partition groups (rows 0:32 and 32:64
    # hold u_m) so each packed-bcat chunk finds its fmap at its base partition
    um2 = [consts.tile([64, 512], BF16, tag=f"um{th}", name=f"um{th}")
           for th in range(TH)]

    # ---------- DMA priority order on the sync queue (bandwidth-bound, so
    # order = arrival time): dx16 -> x16 halves -> wi eighths -> wo.
    # All input tensors are pre-tiled on the host to the exact SBUF layout,
    # so every DMA is a plain contiguous copy (16KB/partition descriptors
    # instead of 2KB gather lines).
    QF = F // 8   # 512 f-columns per wi eighth
    HK = KD // 2
    # dx16 first: the dx correction pass is the PE's gap-filler while the
    # x16 halves are still on the wire
    dx16 = xpool.tile([P, KD * N], BF16, tag="dx16")
    nc.sync.dma_start(out=dx16, in_=io["dxT16"])
    x16h = []
    for h in range(2):
        t = xpool.tile([P, HK * N], BF16, tag=f"x16h{h}", name=f"x16h{h}")
        nc.sync.dma_start(out=t, in_=io["xT16"][h])
        x16h.append(t)
    x16 = [x16h[k // HK][:, (k % HK) * N:(k % HK + 1) * N] for k in range(KD)]
    wi_q = []
    for q in range(8):
        wq = wipool.tile([P, KD * QF], BF16, tag=f"wiq{q}", name=f"wiq{q}")
        nc.sync.dma_start(out=wq, in_=io["wiT"][q])
        wi_q.append(wq)

    def wi_lhsT(k, f):
        q, fr = divmod(f, 4)
        return wi_q[q][:, k * QF + fr * P:k * QF + (fr + 1) * P]

    # ---------- router + lora projection, one fused group ----------
    # [u | logits | dlogits] = (x16 + dx16) @ [Acat | g16 | dg16]:
    # running the same stationary over both x16 and dx16 makes rows 32:40
    # equal x@g16 and rows 64:72 equal x@dg16 (both fp32-accurate), so
    # logits = rows 32:40 + rows 64:72 keeps reference tie-breaks.
    TS = [slice(th * 512, (th + 1) * 512) for th in range(TH)]
    psum_cu = [sps.tile([CW, 512], F32, tag=f"pcu{th}", name=f"pcu{th}")
               for th in range(TH)]
    # dx pass first (dx16 arrives first), x pass second closes the group
    for k in range(KD):
        for th in range(TH):
            nc.tensor.matmul(psum_cu[th], lhsT=cg_sb[k],
                             rhs=dx16[:, k * N:k * N + N][:, TS[th]],
                             start=(k == 0), stop=False)
    for k in range(KD):
        for th in range(TH):
            nc.tensor.matmul(psum_cu[th], lhsT=cg_sb[k],
                             rhs=x16[k][:, TS[th]],
                             start=False, stop=(k == KD - 1))

    # mm1 group helpers. Early groups are emitted th-split (one psum bank
    # each) so up to 6 x-term groups can stay open while the mask chain
    # (DVE/GPSIMD, ~8us) runs — the PE keeps crunching x-terms instead of
    # idling for um; later groups pair th0/th1 per stationary load.
    mm1_ps = {}

    def mm1_x_terms(f, ths):
        for th in ths:
            ps = bps.tile([P, 512], F32, tag="pbig", name=f"p1_{f}_{th}")
            mm1_ps[(f, th)] = ps
            for k in range(KD):
                nc.tensor.matmul(ps, lhsT=wi_lhsT(k, f),
                                 rhs=x16[k][:, TS[th]],
                                 start=(k == 0), stop=False)

    def mm1_x_terms_paired(f):
        ps = [bps.tile([P, 512], F32, tag="pbig", name=f"p1_{f}_{th}")
              for th in range(TH)]
        for th in range(TH):
            mm1_ps[(f, th)] = ps[th]
        for k in range(KD):
            for th in range(TH):
                nc.tensor.matmul(ps[th], lhsT=wi_lhsT(k, f),
                                 rhs=x16[k][:, TS[th]],
                                 start=(k == 0), stop=False)

    def mm1_finish(f, ths=range(TH)):
        q = f // 16
        for th in ths:
            ps = mm1_ps.pop((f, th))
            nc.tensor.matmul(ps, lhsT=bcat_lhsT(f),
                             rhs=um2[th][32 * q:32 * (q + 1), :],
                             start=False, stop=True)
            # activations split across the scalar and vector engines so
            # psum banks recycle at twice the single-engine drain rate
            if th == 0:
                nc.scalar.activation(inter_sb[f][:, TS[0]], ps, Relu,
                                     bias=wib_sb[:, f:f + 1])
            else:
                nc.vector.tensor_scalar(inter_sb[f][:, TS[1]], ps,
                                        wib_sb[:, f:f + 1], 0.0,
                                        mybir.AluOpType.add,
                                        mybir.AluOpType.max)

    # six single-bank x-term groups keep the PE busy through the mask chain
    mm1_x_terms(0, [0, 1])
    mm1_x_terms(1, [0, 1])
    mm1_x_terms(2, [0, 1])

    # ---------- top-1 mask, row-parallel (no transposes) ----------
    # u is copied to SBUF on the scalar engine, which both sidesteps the
    # one-PSUM-operand DVE limit and frees the router psum banks for the
    # mask-replication matmuls below
    u_sb = []
    lgs = []
    for th in range(TH):
        usb = rwork.tile([ER, 512], BF16, tag="usb", name=f"usb{th}")
        nc.scalar.activation(usb, psum_cu[th][0:ER, :],
                             mybir.ActivationFunctionType.Copy)
        u_sb.append(usb)
        lga = rwork.tile([E, 512], F32, tag="lga", name=f"lga{th}")
        nc.vector.tensor_scalar_add(lga, psum_cu[th][ER:ER + E, :],
                                    gateb_col[0:E, :])
        lg = rwork.tile([E, 512], F32, tag="lg", name=f"lg{th}")
        nc.vector.tensor_add(lg, psum_cu[th][64:CW, :], lga)
        lgs.append(lg)

    # ---------- masked lora projection ----------
    # maskT32 = rep.T @ mask8 (tiny matmul), um = u * maskT32
    for th in range(TH):
        mx = rwork.tile([E, 512], F32, tag="mx", name=f"mx{th}")
        nc.gpsimd.partition_all_reduce(mx, lgs[th], channels=E,
                                       reduce_op=bass_isa.ReduceOp.max)
        m8 = rwork.tile([E, 512], BF16, tag="m8", name=f"m8{th}")
        nc.vector.tensor_tensor(m8, lgs[th], mx, mybir.AluOpType.is_equal)
        psum_m = sps.tile([ER, 512], F32, tag=f"pcu{th}", name=f"pm{th}")
        nc.tensor.matmul(psum_m, lhsT=rep, rhs=m8,
                         start=True, stop=True)
        for q in range(2):
            nc.vector.tensor_mul(um2[th][32 * q:32 * (q + 1), :],
                                 psum_m, u_sb[th])

    # ---------- matmul 1: interT = relu(wi @ x.T + Bcat.T @ u_m + wi_b) ------
    for f in range(3):
        mm1_finish(f)
    for f in range(3, KF):
        mm1_x_terms_paired(f)
        mm1_finish(f)

    # ---------- matmul 2: outT = wo @ inter + wo_b ----------
    # woT [F, D] column-block d fetched as ONE 3D DMA into [p, (kf j)] layout:
    # wo_big[p, kf*128 + j] = woT[kf*128 + p, d*128 + j]
    for d in range(KD):
        wo_big = wop.tile([P, F], BF16, tag="wo", name=f"wo{d}")
        nc.sync.dma_start(out=wo_big, in_=io["woTt"][d])
        ps = [bps.tile([P, 512], F32, tag="pbig", name=f"p2_{d}_{th}")
              for th in range(TH)]
        for kf in range(KF):
            for th in range(TH):
                nc.tensor.matmul(ps[th], lhsT=wo_big[:, kf * P:(kf + 1) * P],
                                 rhs=inter_sb[kf][:, TS[th]],
                                 start=(kf == 0), stop=(kf == KF - 1))
        osb = outp.tile([P, N], F32, tag="osb")
        if d < KD - 1:
            for th in range(TH):
                nc.vector.tensor_scalar(osb[:, TS[th]], ps[th],
                                        wob_sb[:, d:d + 1], None,
                                        mybir.AluOpType.add)
            nc.gpsimd.dma_start(out=io["outT"][d * P:(d + 1) * P, :], in_=osb)
        else:
            # last d-tile: store each half as soon as its psum closes so the
            # th0 transfer overlaps the final matmuls and the tail transfer
            # is half-sized; ride the sync queue (hardware DGE, idle and
            # fully drained by now) for lower completion latency
            for th in range(TH):
                nc.vector.tensor_scalar(osb[:, TS[th]], ps[th],
                                        wob_sb[:, d:d + 1], None,
                                        mybir.AluOpType.add)
                nc.sync.dma_start(out=io["outT"][d * P:(d + 1) * P, TS[th]],
                                  in_=osb[:, TS[th]])


_CACHED_NC = None


def build_nc():
    global _CACHED_NC
    if _CACHED_NC is not None:
        return _CACHED_NC
    nc = bacc.Bacc("TRN2", target_bir_lowering=False, debug=False,
                   enable_asserts=False, num_devices=NCORES)
    decls = [
        ("xT16", [2, P, (KD // 2) * N], BF16, False),
        ("dxT16", [P, KD * N], BF16, False),
        ("cgT", [P, KD * 72], BF16, False),
        ("repM", [E, ER], BF16, False),
        ("biases", [P, 49], F32, False),
        ("bT", [64, F // 2], BF16, False),
        ("wiT", [8, P, KD * (F // 8)], BF16, False),
        ("woTt", [KD, P, F], BF16, False),
        ("outT", [D, N], F32, True),
    ]
    io = {}
    for name, shape, dt_, is_out in decls:
        io[name] = nc.dram_tensor(
            name, shape, dt_, kind="ExternalOutput" if is_out else "ExternalInput"
        ).ap()
    with tile.TileContext(nc) as tc:
        with ExitStack() as ctx:
            _emit(ctx, tc, io)
    nc.compile()
    _CACHED_NC = nc
    return nc


def make_in_maps(inputs: dict) -> list[dict]:
    f32 = np.float32
    x = np.ascontiguousarray(np.asarray(inputs["hidden_states"], f32).reshape(NT, D))
    gT = np.asarray(inputs["gate_W"], f32).T                                # [D, E]
    g16 = gT.astype(BF)
    dg16 = (gT - g16.astype(f32)).astype(BF)
    aT = np.asarray(inputs["lora_A"], f32).reshape(ER, D).T                 # [D, 32]
    cgT_flat = np.concatenate(
        [aT.astype(BF), g16, np.zeros((D, 24), BF), dg16], axis=1)          # [D, 72]
    cgT = np.ascontiguousarray(
        cgT_flat.reshape(KD, P, 72).transpose(1, 0, 2).reshape(P, KD * 72))
    biases = np.zeros((P, 49), f32)
    biases[:, 0:KF] = np.asarray(inputs["wi_b"], f32).reshape(KF, P).T
    biases[:, KF:KF + KD] = np.asarray(inputs["wo_b"], f32).reshape(KD, P).T
    biases[:, KF + KD:KF + KD + E] = np.asarray(inputs["gate_b"], f32)[None, :]
    biases[0:E, 48] = np.asarray(inputs["gate_b"], f32)
    bT_flat = np.asarray(inputs["lora_B"], f32).transpose(0, 2, 1).reshape(ER, F)
    # packed [64, F/2]: rows 32q:32q+32 = Bcat.T columns q*2048:(q+1)*2048
    bT = np.ascontiguousarray(
        bT_flat.reshape(ER, 2, F // 2).transpose(1, 0, 2).reshape(64, F // 2).astype(BF))
    # wi pre-tiled per eighth: wiT[q, p, k*512 + f] = wi_W.T[k*128+p, q*512+f]
    wiT = np.ascontiguousarray(
        np.asarray(inputs["wi_W"], f32).T.astype(BF)
        .reshape(KD, P, 8, F // 8).transpose(2, 1, 0, 3).reshape(8, P, KD * (F // 8)))
    woT = np.asarray(inputs["wo_W"], f32).T.astype(BF)                      # [F, D]
    # pre-tiled to SBUF layout: woTt[d, p, kf*128+j] = woT[kf*128+p, d*128+j]
    woTt = np.ascontiguousarray(
        woT.reshape(KF, P, KD, P).transpose(2, 1, 0, 3).reshape(KD, P, F))
    repM = np.ascontiguousarray(
        (np.arange(E)[:, None] == (np.arange(ER)[None, :] // R)).astype(BF))

    in_maps = []
    for c in range(NCORES):
        xT32 = x[c * N:(c + 1) * N].T                                       # [D, N]
        xT16 = xT32.astype(BF)
        dxT16 = (xT32 - xT16.astype(f32)).astype(BF)
        # pre-tiled: xT16[h, p, k4*1024 + t] = x.T[(h*4+k4)*128 + p, t]
        xT16t = np.ascontiguousarray(
            xT16.reshape(2, KD // 2, P, N).transpose(0, 2, 1, 3)
            .reshape(2, P, (KD // 2) * N))
        dxT16t = np.ascontiguousarray(
            dxT16.reshape(KD, P, N).transpose(1, 0, 2).reshape(P, KD * N))
        in_maps.append({
            "xT16": xT16t,
            "dxT16": dxT16t,
            "cgT": cgT, "repM": repM, "biases": biases, "bT": bT,
            "wiT": wiT, "woTt": woTt,
        })
    return in_maps


def kernel(**inputs) -> np.ndarray:
    nc = build_nc()
    in_maps = make_in_maps(inputs)
    res = run_bass_kernel_spmd(nc, in_maps, core_ids=list(range(NCORES)))
    out = np.empty((NT, D), np.float32)
    for c in range(NCORES):
        out[c * N:(c + 1) * N] = res.results[c]["outT"].T
    return out.reshape(B, S, D)
